# revision 1
# baseline (speedup 1.0000x reference)
"""Trainium2 Bass kernel for nn_Attention_26628797235884.

12-head attention block (qkv proj + per-head RMS norm + 2D RoPE + softmax
attention + output proj), batch 8 x seq 1024 x dim 768, distributed
data-parallel over batch across 8 NeuronCores (batch b -> core b, weights
replicated, no collectives).

Per-core layout strategy:
  - q,k computed feature-major ("transposed", [feat, seq]) so the QK^T and
    PV matmuls contract along partitions without on-device transposes.
  - RMS norm + RoPE fused into a few DVE passes; sum-of-squares group
    reduction done on the TensorEngine with a 0/1 indicator matrix.
  - scores computed transposed [sk, sq]; softmax denominator obtained for
    free by appending a ones-column to V (M=65 PV matmul); softmax has no
    max-subtraction (logits are O(1) for RMS-normed q,k).
  - v and the final projection computed in natural [seq, feat] layout.
All matmuls run in bf16 (inputs pre-cast on host), fp32 accumulation.
"""

import sys

import numpy as np
import ml_dtypes

try:
    import concourse.bass as bass  # noqa: F401
except ImportError:  # pragma: no cover
    sys.path.insert(0, "/opt/trn_rl_repo")

import concourse.tile as tile
from concourse import bacc, mybir
from concourse.bass_utils import run_bass_kernel_spmd

BF16 = mybir.dt.bfloat16
F32 = mybir.dt.float32
AF = mybir.ActivationFunctionType
OP = mybir.AluOpType
NP_BF16 = ml_dtypes.bfloat16

B, S, C, H, D = 8, 1024, 768, 12, 64
KT = C // 128          # 6 contraction tiles over the model dim
ST = S // 128          # 8 seq tiles
NCORES = 8
EPS = 1e-6
PAIRSWAP32 = [i ^ 1 for i in range(32)]

_CACHE = {}
DEBUG = False  # adds intermediate-tensor outputs to the graph (dev only)


# --------------------------------------------------------------------------
# host-side constant prep
# --------------------------------------------------------------------------

def _rope_tables():
    ROPE_DIM, PT_SEQ, FT_SEQ, THETA = 32, 16, 32, 10000.0
    freqs = 1.0 / (THETA ** (np.arange(0, ROPE_DIM, 2, dtype=np.float32)[: ROPE_DIM // 2] / ROPE_DIM))
    t = np.arange(FT_SEQ, dtype=np.float32) / FT_SEQ * PT_SEQ
    f = np.einsum("i,j->ij", t, freqs)
    f = np.repeat(f, 2, axis=-1)
    fh = np.broadcast_to(f[:, None, :], (FT_SEQ, FT_SEQ, ROPE_DIM))
    fw = np.broadcast_to(f[None, :, :], (FT_SEQ, FT_SEQ, ROPE_DIM))
    f2 = np.concatenate([fh, fw], axis=-1).reshape(FT_SEQ * FT_SEQ, 2 * ROPE_DIM)
    return np.cos(f2).astype(np.float32), np.sin(f2).astype(np.float32)


def _prep_shared(qkv_w, qkv_b, q_norm_w, k_norm_w, proj_w, proj_b):
    f32 = np.float32
    cos, sin = _rope_tables()                 # [S, D]
    pair = np.arange(D) ^ 1
    sa = sin.copy()
    sa[:, 0::2] *= -1.0                       # sign-folded sin for rotate_half

    def mk(tab, w):                           # -> [128, S] bf16, 2 heads stacked
        t = (tab * w[None, :]).T.astype(f32)  # [64, S]
        return np.ascontiguousarray(np.vstack([t, t])).astype(NP_BF16)

    qw = np.asarray(q_norm_w, f32)
    kw = np.asarray(k_norm_w, f32)
    shared = {
        "wqkT": np.ascontiguousarray(np.asarray(qkv_w, f32)[: 2 * C].T).astype(NP_BF16),
        "wvT": np.ascontiguousarray(np.asarray(qkv_w, f32)[2 * C :].T).astype(NP_BF16),
        "pwT": np.ascontiguousarray(np.asarray(proj_w, f32).T).astype(NP_BF16),
        "cosq": mk(cos, qw),
        "sinq": mk(sa, qw[pair]),
        "cosk": mk(cos, kw),
        "sink": mk(sa, kw[pair]),
    }
    b = np.asarray(qkv_b, f32)
    bqk = np.ascontiguousarray(b[: 2 * C].reshape(2 * KT, 128).T)        # [128, 12]
    shared["bqk"] = bqk
    shared["bqks"] = np.ascontiguousarray(bqk[np.arange(128) ^ 1, :])
    shared["vbias"] = np.ascontiguousarray(np.tile(b[2 * C :][None, :], (128, 1)))
    shared["pbias"] = np.ascontiguousarray(np.tile(np.asarray(proj_b, f32)[None, :], (128, 1)))
    # E_big[:, 10 - 2i : 22 - 2i] is a [128, 12] indicator lhsT whose column
    # 2i+g selects partition half g — lets 6 m-tiles' group-sums accumulate
    # into one [12, S] PSUM tensor.
    ebig = np.zeros((128, 22), NP_BF16)
    ebig[0:64, 10] = 1
    ebig[64:128, 11] = 1
    shared["ebig"] = ebig
    # sel[:, 128i:128i+128] broadcasts rinv rows (2i, 2i+1) to the 2 head
    # halves of a [128, S] field via lhsT.T @ rinv (partition_broadcast to
    # base partition 64 silently no-ops on HW, so fields go through PE).
    sel = np.zeros((12, 6 * 128), NP_BF16)
    for i in range(6):
        sel[2 * i, 128 * i : 128 * i + 64] = 1
        sel[2 * i + 1, 128 * i + 64 : 128 * i + 128] = 1
    shared["sel"] = sel
    return shared


# --------------------------------------------------------------------------
# device graph
# --------------------------------------------------------------------------

def _graph(tc, d, out_d, dbg=None):
    nc = tc.nc
    from contextlib import ExitStack

    with ExitStack() as big:
        main = big.enter_context(tc.tile_pool(name="main", bufs=1))
        workp = big.enter_context(tc.tile_pool(name="workp", bufs=2))

        pwT = main.tile([128, KT, C], BF16, tag="pwT")
        nc.sync.dma_start(pwT[:], d["pwT"].rearrange("(k p) o -> p k o", p=128))
        pbias = main.tile([128, C], F32, tag="pbias")
        nc.sync.dma_start(pbias[:], d["pbias"][:])
        ebig = main.tile([128, 22], BF16, tag="ebig")
        nc.sync.dma_start(ebig[:], d["ebig"][:])
        sel = main.tile([12, 6 * 128], BF16, tag="sel")
        nc.sync.dma_start(sel[:], d["sel"][:])
        qk_sb = [main.tile([128, S], BF16, tag=f"qk{m}", name=f"qk{m}") for m in range(2 * KT)]
        v_sb = [main.tile([128, H * 65], BF16, tag=f"v{j}", name=f"v{j}") for j in range(ST)]
        outT = [main.tile([128, S], BF16, tag=f"ot{p}", name=f"ot{p}") for p in range(KT)]

        # ---------------- stage 1+2: projections, norm, rope ----------------
        with ExitStack() as early:
            ep = early.enter_context(tc.tile_pool(name="early", bufs=1))
            w1 = early.enter_context(tc.tile_pool(name="w1", bufs=2))
            w1b = early.enter_context(tc.tile_pool(name="w1b", bufs=6))
            t1p = early.enter_context(tc.tile_pool(name="t1p", bufs=3))
            ps_mm = early.enter_context(tc.tile_pool(name="psmm", bufs=5, space="PSUM"))
            ps_sq = early.enter_context(tc.tile_pool(name="pssq", bufs=1, space="PSUM"))
            ps_fld = early.enter_context(tc.tile_pool(name="psfld", bufs=1, space="PSUM"))

            # split per k-tile so the first QKV matmuls start after ~1/6 of
            # the weight load instead of the whole 5MB
            xT = ep.tile([128, KT, S], BF16, tag="xT")
            xT_r = d["xT"].rearrange("(k p) s -> p k s", p=128)
            wqkT = ep.tile([128, KT, 2 * C], BF16, tag="wqkT")
            wqkT_r = d["wqkT"].rearrange("(k p) o -> p k o", p=128)
            wvT = ep.tile([128, KT, C], BF16, tag="wvT")
            wvT_r = d["wvT"].rearrange("(k p) o -> p k o", p=128)
            for k in range(KT):
                nc.sync.dma_start(xT[:, k], xT_r[:, k])
                nc.sync.dma_start(wqkT[:, k], wqkT_r[:, k])
            for k in range(KT):
                nc.sync.dma_start(wvT[:, k], wvT_r[:, k])
            tabs = {}
            for nm in ("cosq", "sinq", "cosk", "sink"):
                tabs[nm] = ep.tile([128, S], BF16, tag=nm, name=nm)
                nc.sync.dma_start(tabs[nm][:], d[nm][:])
            bqk = ep.tile([128, 2 * KT], F32, tag="bqk")
            nc.sync.dma_start(bqk[:], d["bqk"][:])
            bqks = ep.tile([128, 2 * KT], F32, tag="bqks")
            nc.sync.dma_start(bqks[:], d["bqks"][:])
            vbias = ep.tile([128, C], F32, tag="vbias")
            nc.sync.dma_start(vbias[:], d["vbias"][:])

            # two batches of 6 m-tiles (q-tile p and k-tile 6+p interleaved so
            # early head-pairs complete first). All stage-1 PSUM traffic is
            # 512-wide (1 bank) so slots recycle fast and the PE stays dense.
            def emit_vproj():
                for j in range(ST):
                    vt = v_sb[j]
                    vview = vt[:].rearrange("p (h e) -> p h e", e=65)
                    nc.gpsimd.memset(vview[:, :, 64:65], 1.0)
                    for h2, (lo, hi) in enumerate(((0, 512), (512, 768))):
                        ps = ps_mm.tile([128, 512], F32, tag="mm", name=f"vmm{j}_{h2}")
                        w = hi - lo
                        for k in range(KT):
                            nc.tensor.matmul(
                                ps[:, 0:w], xT[:, k, 128 * j : 128 * j + 128], wvT[:, k, lo:hi],
                                start=(k == 0), stop=(k == KT - 1),
                            )
                        nc.vector.tensor_add(
                            vview[:, 8 * h2 : 8 * h2 + w // 64, 0:64].rearrange("p h e -> p (h e)") if False else
                            vt[:].rearrange("p (h e) -> p h e", e=65)[:, lo // 64 : hi // 64, 0:64],
                            ps[:, 0:w].rearrange("p (h dd) -> p h dd", dd=64),
                            vbias[:, lo:hi].rearrange("p (h dd) -> p h dd", dd=64),
                        )
                    if dbg is not None:
                        nc.sync.dma_start(dbg[f"v{j}"][:], vt[:])

            m_order = [0, 6, 1, 7, 2, 8, 3, 9, 4, 10, 5, 11]
            batches = [m_order[0:6], m_order[6:12]]
            pending_tails = []
            for batch, ms in enumerate(batches):
                nb = len(ms)
                sqb = ps_sq.tile([2 * nb, S], F32, tag="sq", name=f"sqb{batch}")
                t1s = []
                for i, m in enumerate(ms):
                    if batch >= 1 and i == 2 and pending_tails:
                        pending_tails.pop(0)()  # previous batch's fields after this one ramps up
                    ctab = tabs["cosq"] if m < KT else tabs["cosk"]
                    stab = tabs["sinq"] if m < KT else tabs["sink"]
                    t1 = t1p.tile([128, S], BF16, tag=f"t1_{i}", name=f"t1_{batch}_{i}")
                    for h2 in range(2):
                        cs = slice(512 * h2, 512 * h2 + 512)
                        ps = ps_mm.tile([128, 512], F32, tag="mm", name=f"mm{batch}_{i}_{h2}")
                        for k in range(KT):
                            nc.tensor.matmul(
                                ps[:],
                                wqkT[:, k, 128 * m : 128 * m + 128],
                                xT[:, k, cs],
                                start=(k == 0),
                                stop=(k == KT - 1),
                            )
                        # single PSUM reader: t = (ps + b) -> bf16 SBUF on ACT,
                        # so the PSUM slot recycles after one read and all the
                        # rope/square math runs in DVE 2x bf16 mode
                        t = w1b.tile([128, 512], BF16, tag="t")
                        nc.scalar.activation(t[:], ps[:], AF.Identity, bias=bqk[:, m : m + 1], scale=1.0)
                        t2 = w1b.tile([128, 512], BF16, tag="t2")
                        nc.vector.tensor_mul(t2[:], t[:], t[:])
                        nc.tensor.matmul(
                            sqb[:, cs],
                            ebig[:, 10 - 2 * i : 10 - 2 * i + 2 * nb],
                            t2[:],
                            start=(i == 0), stop=(i == nb - 1),
                        )
                        # rope: u = t*cos ; v = shuffle(t)*sinA (bias already in t)
                        u = w1b.tile([128, 512], BF16, tag="u")
                        nc.vector.tensor_mul(u[:], t[:], ctab[:, cs])
                        tsh = w1b.tile([128, 512], BF16, tag="tsh")
                        nc.vector.stream_shuffle(tsh[:], t[:], PAIRSWAP32)
                        vv = w1b.tile([128, 512], BF16, tag="vv")
                        nc.vector.tensor_mul(vv[:], tsh[:], stab[:, cs])
                        nc.gpsimd.tensor_add(t1[:, cs], u[:], vv[:])
                    t1s.append(t1)
                # rinv = 1/sqrt(ssq/64 + eps) for the whole batch
                epsc = w1.tile([2 * nb, 1], F32, tag="epsc", name=f"epsc{batch}")
                nc.gpsimd.memset(epsc[:], EPS)
                rms = w1.tile([2 * nb, S], F32, tag="rms", name=f"rms{batch}")
                nc.scalar.activation(rms[:], sqb[:], AF.Sqrt, bias=epsc[:], scale=1.0 / D)
                rinv = w1.tile([2 * nb, S], F32, tag="rinv", name=f"rinv{batch}")
                nc.vector.reciprocal_approx_fast(rinv[:], rms[:])
                rinv_bf = w1.tile([2 * nb, S], BF16, tag="rinv_bf", name=f"rinvbf{batch}")
                nc.vector.tensor_copy(rinv_bf[:], rinv[:])
                def _mk_tail(ms=ms, t1s=t1s, rinv_bf=rinv_bf, nb=nb):
                    def _tail():
                        for i, m in enumerate(ms):
                            for h2 in range(2):
                                cs = slice(512 * h2, 512 * h2 + 512)
                                fldp = ps_fld.tile([128, 512], F32, tag="fld", name=f"fld{m}_{h2}")
                                nc.tensor.matmul(
                                    fldp[:],
                                    sel[0 : 2 * nb, 128 * i : 128 * i + 128],
                                    rinv_bf[:, cs],
                                    start=True, stop=True,
                                )
                                nc.vector.tensor_mul(qk_sb[m][:, cs], t1s[i][:, cs], fldp[:])
                            if dbg is not None:
                                nc.sync.dma_start(dbg[f"qk{m}"][:], qk_sb[m][:])
                    return _tail
                pending_tails.append(_mk_tail())
                if batch == 0:
                    emit_vproj()

            # ---------------- stage 2: V projection (natural layout) --------
            for t in pending_tails:
                t()
            pending_tails = []

        # ---------------- stage 3: attention, one head-pair at a time -------
        # scores/exp per head so scores(j+1) overlaps exp(j) via sc double-buffer
        with ExitStack() as att:
            xp = att.enter_context(tc.tile_pool(name="attx", bufs=20))
            af = att.enter_context(tc.tile_pool(name="attf", bufs=2))
            ps_sc = att.enter_context(tc.tile_pool(name="pssc", bufs=2, space="PSUM"))
            ps_pv = att.enter_context(tc.tile_pool(name="pspv", bufs=2, space="PSUM"))

            pending = None  # deferred PV emission: (p, j, eA, eB, pvA, pvB)

            def emit_pv(p, j, eA, eB, pvA, pvB):
                hA, hB = 2 * p, 2 * p + 1
                vva = v_sb[j][:].rearrange("p (h e) -> p h e", e=65)
                for h2 in range(2):
                    nc.tensor.matmul(
                        pvA[0:65, 512 * h2 : 512 * h2 + 512],
                        vva[:, hA, :],
                        eA[:, 512 * h2 : 512 * h2 + 512],
                        start=(j == 0), stop=(j == ST - 1),
                    )
                    nc.tensor.matmul(
                        pvB[0:65, 512 * h2 : 512 * h2 + 512],
                        vva[:, hB, :],
                        eB[:, 512 * h2 : 512 * h2 + 512],
                        start=(j == 0), stop=(j == ST - 1),
                    )

            for p in range(KT):
                qt, kt = qk_sb[p], qk_sb[KT + p]
                pvA = ps_pv.tile([128, S], F32, tag="pv", name=f"pvA{p}")
                pvB = ps_pv.tile([128, S], F32, tag="pv", name=f"pvB{p}")
                for j in range(ST):
                    # scores(j) + exp(j) go into the engine streams BEFORE
                    # PV(j-1) so the PE never queues behind an exp wait
                    scA = ps_sc.tile([128, S], F32, tag="sc", name=f"scA{p}_{j}")
                    scB = ps_sc.tile([128, S], F32, tag="sc", name=f"scB{p}_{j}")
                    for h2 in range(2):
                        nc.tensor.matmul(
                            scA[:, 512 * h2 : 512 * h2 + 512],
                            kt[0:64, 128 * j : 128 * j + 128],
                            qt[0:64, 512 * h2 : 512 * h2 + 512],
                            start=True, stop=True,
                        )
                        nc.tensor.matmul(
                            scB[:, 512 * h2 : 512 * h2 + 512],
                            kt[64:128, 128 * j : 128 * j + 128],
                            qt[64:128, 512 * h2 : 512 * h2 + 512],
                            start=True, stop=True,
                        )
                    eA = xp.tile([128, S], BF16, tag="exp", name=f"eA{p}_{j}")
                    nc.scalar.activation(eA[:], scA[:], AF.Exp, scale=0.125)
                    eB = xp.tile([128, S], BF16, tag="exp", name=f"eB{p}_{j}")
                    nc.scalar.activation(eB[:], scB[:], AF.Exp, scale=0.125)
                    if pending is not None:
                        emit_pv(*pending)
                    pending = (p, j, eA, eB, pvA, pvB)
                # flush the last deferred PV before the norm reads the pv tiles
                emit_pv(*pending)
                pending = None
                # softmax denominators live in row 64; DVE cannot move data
                # across partitions and DMA cannot read PSUM, so copy the row
                # out at base 64 and DMA it down to partition 0/1.
                rsxA = af.tile([65, S], F32, tag="rsxA")
                nc.vector.tensor_copy(rsxA[64:65, :], pvA[64:65, :])
                rsxB = af.tile([65, S], F32, tag="rsxB")
                nc.vector.tensor_copy(rsxB[64:65, :], pvB[64:65, :])
                rr = af.tile([2, S], F32, tag="rr")
                nc.sync.dma_start(rr[0:1, :], rsxA[64:65, :])
                nc.sync.dma_start(rr[1:2, :], rsxB[64:65, :])
                ri = af.tile([2, S], F32, tag="ri")
                nc.vector.reciprocal_approx_fast(ri[:], rr[:])
                rfixB = af.tile([1, S], F32, tag="rfixB")
                nc.sync.dma_start(rfixB[:], ri[1:2, :])
                fldA = af.tile([64, S], F32, tag="fldA")
                nc.gpsimd.partition_broadcast(fldA[:], ri[0:1, :])
                fldB = af.tile([64, S], F32, tag="fldB")
                nc.gpsimd.partition_broadcast(fldB[:], rfixB[0:1, :])
                nc.vector.tensor_mul(outT[p][0:64, :], pvA[0:64, :], fldA[:])
                tmpB = af.tile([64, S], BF16, tag="tmpB")
                nc.vector.tensor_mul(tmpB[:], pvB[0:64, :], fldB[:])
                nc.sync.dma_start(outT[p][64:128, :], tmpB[:])
                if dbg is not None:
                    nc.sync.dma_start(dbg[f"ot{p}"][:], outT[p][:])
                    nc.sync.dma_start(dbg[f"rr{p}"][:], rr[:])

        # ---------------- stage 4: output projection ------------------------
        with tc.tile_pool(name="psy", bufs=2, space="PSUM") as ps_y:
            for mt in range(ST):
                ps = ps_y.tile([128, C], F32, tag="y")
                for k6 in range(KT):
                    nc.tensor.matmul(
                        ps[:, 0:512], outT[k6][:, 128 * mt : 128 * mt + 128], pwT[:, k6, 0:512],
                        start=(k6 == 0), stop=(k6 == KT - 1),
                    )
                    nc.tensor.matmul(
                        ps[:, 512:768], outT[k6][:, 128 * mt : 128 * mt + 128], pwT[:, k6, 512:768],
                        start=(k6 == 0), stop=(k6 == KT - 1),
                    )
                y = workp.tile([128, C], F32, tag="y_sb")
                nc.vector.tensor_add(y[:], ps[:], pbias[:])
                nc.sync.dma_start(out_d[128 * mt : 128 * mt + 128, :], y[:])



LDW_OPT = False  # walrus LDW-opt rejects bass InstLdweights


def _patch_walrus():
    import concourse.bass_utils as _bu
    if getattr(_bu, "_ldwopt_patched", False):
        return
    _orig = _bu.run_command

    def _patched(cmd, **kw):
        if LDW_OPT and isinstance(cmd, list):
            cmd = ["--enable-ldw-opt=true" if c == "--enable-ldw-opt=false" else c for c in cmd]
        return _orig(cmd, **kw)

    _bu.run_command = _patched
    _bu._ldwopt_patched = True


def build():
    if "nc" in _CACHE:
        return _CACHE["nc"]
    _patch_walrus()
    nc = bacc.Bacc("TRN2", target_bir_lowering=False, debug=False)
    d = {}

    def din(name, shape, dt):
        d[name] = nc.dram_tensor(name, shape, dt, kind="ExternalInput").ap()

    din("xT", [C, S], BF16)
    din("wqkT", [C, 2 * C], BF16)
    din("wvT", [C, C], BF16)
    din("pwT", [C, C], BF16)
    din("bqk", [128, 2 * KT], F32)
    din("bqks", [128, 2 * KT], F32)
    din("vbias", [128, C], F32)
    din("pbias", [128, C], F32)
    din("cosq", [128, S], BF16)
    din("sinq", [128, S], BF16)
    din("cosk", [128, S], BF16)
    din("sink", [128, S], BF16)
    din("ebig", [128, 22], BF16)
    din("sel", [12, 6 * 128], BF16)
    out_d = nc.dram_tensor("out", [S, C], F32, kind="ExternalOutput").ap()
    dbg = None
    if DEBUG:
        dbg = {}
        for m in range(2 * KT):
            dbg[f"qk{m}"] = nc.dram_tensor(f"dbg_qk{m}", [128, S], BF16, kind="ExternalOutput").ap()
        for j in range(ST):
            dbg[f"v{j}"] = nc.dram_tensor(f"dbg_v{j}", [128, H * 65], BF16, kind="ExternalOutput").ap()
        for p in range(KT):
            dbg[f"ot{p}"] = nc.dram_tensor(f"dbg_ot{p}", [128, S], BF16, kind="ExternalOutput").ap()
            dbg[f"rr{p}"] = nc.dram_tensor(f"dbg_rr{p}", [2, S], F32, kind="ExternalOutput").ap()
        dbg["exp00"] = nc.dram_tensor("dbg_exp00", [128, 2048], BF16, kind="ExternalOutput").ap()

    with tile.TileContext(nc) as tc:
        _graph(tc, d, out_d, dbg)
    nc.compile()
    _CACHE["nc"] = nc
    return nc


def make_in_maps(x, qkv_w, qkv_b, q_norm_w, k_norm_w, proj_w, proj_b):
    shared = _prep_shared(qkv_w, qkv_b, q_norm_w, k_norm_w, proj_w, proj_b)
    x = np.asarray(x, np.float32)
    in_maps = []
    for b in range(NCORES):
        m = dict(shared)
        m["xT"] = np.ascontiguousarray(x[b].T).astype(NP_BF16)
        in_maps.append(m)
    return in_maps


def run(in_maps, trace=False, **kw):
    nc = build()
    return run_bass_kernel_spmd(nc, in_maps, core_ids=list(range(NCORES)), trace=trace, **kw)


def kernel(x, qkv_w, qkv_b, q_norm_w, k_norm_w, proj_w, proj_b):
    in_maps = make_in_maps(x, qkv_w, qkv_b, q_norm_w, k_norm_w, proj_w, proj_b)
    res = run(in_maps)
    return np.stack([np.asarray(res.results[i]["out"]) for i in range(NCORES)]).astype(np.float32)


if __name__ == "__main__":
    rng = np.random.default_rng(0)
    ins = {
        "x": rng.standard_normal((B, S, C)).astype(np.float32),
        "qkv_w": (rng.standard_normal((3 * C, C)) * C**-0.5).astype(np.float32),
        "qkv_b": (rng.standard_normal(3 * C) * 0.02).astype(np.float32),
        "q_norm_w": np.ones(D, np.float32),
        "k_norm_w": np.ones(D, np.float32),
        "proj_w": (rng.standard_normal((C, C)) * C**-0.5).astype(np.float32),
        "proj_b": (rng.standard_normal(C) * 0.02).astype(np.float32),
    }
    y = kernel(**ins)
    print("out", y.shape, y.dtype)

